# revision 1
# baseline (speedup 1.0000x reference)
# Trainium2 Bass kernel for ChannelAttentionBlock (B=8,C=256,H=W=128,S=64,HEADS=8)
# Data-parallel over batch: 1 sample per NeuronCore, 8 cores.
import numpy as np
import ml_dtypes

import concourse.bass as bass
from concourse import bacc
import concourse.mybir as mybir
from concourse.bass_utils import run_bass_kernel_spmd
from concourse.tile import TileContext

F32R = mybir.dt.float32r
F32 = mybir.dt.float32
BF16 = mybir.dt.bfloat16
AF = mybir.ActivationFunctionType
ALU = mybir.AluOpType

B, C, H, W = 8, 256, 128, 128
S = 64
HEADS = 8
HW = H * W
WP = W + 2          # padded row width
R = 16              # rows per strip
NSTRIP = H // R
BLK_ROWS = 4
NBLK = 4
NPX = BLK_ROWS * W  # 512
EPS = 1e-5

TAPS = [(dy, dx) for dy in (-1, 0, 1) for dx in (-1, 0, 1)]

_CACHED = {}


def build_nc(debug_taps=False):
    nc = bacc.Bacc("TRN2", target_bir_lowering=False, debug=False)

    # ------------- DRAM parameters (host layouts) -------------
    # x/y: [128(part), 2(ktile), H+4(rows: img rows -2..129), WP]
    x_in = nc.dram_tensor("x", [128, 2, H + 4, WP], F32R, kind="ExternalInput")
    y_in = nc.dram_tensor("y", [128, 2, H + 4, WP], F32R, kind="ExternalInput")
    wcq_in = nc.dram_tensor("wcq", [128, 2, 9, S + 1], F32R, kind="ExternalInput")
    wckv_in = nc.dram_tensor("wckv", [128, 2, 9, S + 1], F32R, kind="ExternalInput")
    wqkv_in = nc.dram_tensor("wqkv", [128, 9, 128], F32R, kind="ExternalInput")
    wpo_in = nc.dram_tensor("wpo", [S, S], F32R, kind="ExternalInput")
    wexp_in = nc.dram_tensor("wexp", [S, 9, C], BF16, kind="ExternalInput")
    wf1x_in = nc.dram_tensor("wf1x", [128, 2, 2, 128], F32R, kind="ExternalInput")
    wf1v_in = nc.dram_tensor("wf1v", [128, 2, 2, 128], F32R, kind="ExternalInput")
    wdw_in = nc.dram_tensor("wdw", [128, 2, 9, 128], F32R, kind="ExternalInput")
    wf2_in = nc.dram_tensor("wf2", [128, 2, 2, 128], F32R, kind="ExternalInput")
    stat_cq_in = nc.dram_tensor("stat_cq", [S + 1, 1], F32R, kind="ExternalInput")
    bc2a_in = nc.dram_tensor("bc2a", [1, 128], F32R, kind="ExternalInput")    # 1s at rows0-63
    bc2b_in = nc.dram_tensor("bc2b", [1, 128], F32R, kind="ExternalInput")    # 1s at rows64-127
    ones128_in = nc.dram_tensor("ones128", [1, 128], F32R, kind="ExternalInput")
    stat256_in = nc.dram_tensor("stat256", [128, 2], F32R, kind="ExternalInput")
    bias_q_in = nc.dram_tensor("bias_q", [S, 1], F32, kind="ExternalInput")
    bias_kv_in = nc.dram_tensor("bias_kv", [2 * S, 1], F32, kind="ExternalInput")
    bias_g_in = nc.dram_tensor("bias_g", [128, 2, 1], F32, kind="ExternalInput")
    temp_in = nc.dram_tensor("tempv", [S, 1], F32, kind="ExternalInput")
    mask_in = nc.dram_tensor("maskbd", [S, S], F32R, kind="ExternalInput")
    ident_in = nc.dram_tensor("ident", [128, 128], BF16, kind="ExternalInput")
    identr_in = nc.dram_tensor("identr", [S, S], F32R, kind="ExternalInput")
    ones_in = nc.dram_tensor("onesr", [1, S], F32R, kind="ExternalInput")
    zeros_in = nc.dram_tensor("zeros", [128, 2600], F32R, kind="ExternalInput")
    zerosb_in = nc.dram_tensor("zerosb", [128, 2600], BF16, kind="ExternalInput")

    fx_dram = nc.dram_tensor("fx_dram", [2, 128, HW], BF16)
    out_dram = nc.dram_tensor("out", [2, 128, HW], F32, kind="ExternalOutput")
    if debug_taps:
        dbg_xq = nc.dram_tensor("dbg_xq", [128, HW], F32, kind="ExternalOutput")
        dbg_qk = nc.dram_tensor("dbg_qk", [128, HW], F32, kind="ExternalOutput")
        dbg_vo = nc.dram_tensor("dbg_vo", [128, HW], F32, kind="ExternalOutput")
        dbg_at = nc.dram_tensor("dbg_at", [S, S + 2], F32, kind="ExternalOutput")
        dbg_f1 = nc.dram_tensor("dbg_f1", [128, 2, HW], F32, kind="ExternalOutput")

    with TileContext(nc) as tc:
        with tc.tile_pool(name="persist", bufs=1) as persist:
            qk_store = persist.tile([128, HW], BF16, tag="qk_store")
            vo_store = persist.tile([128, HW], BF16, tag="vo_store")
            rq2 = persist.tile([S, 1], F32, tag="rq2")
            rk2 = persist.tile([S, 1], F32, tag="rk2")
            nc.vector.memset(rq2, 0.0)
            nc.vector.memset(rk2, 0.0)

            # ================= PHASE 1 =================
            with tc.tile_pool(name="p1w", bufs=1) as p1w, \
                 tc.tile_pool(name="p1", bufs=2) as p1, \
                 tc.tile_pool(name="p1ps", bufs=1, space="PSUM") as p1ps:
                wcq = p1w.tile([128, 2, 9, S + 1], F32R, tag="wcq")
                nc.sync.dma_start(out=wcq, in_=wcq_in[:, :, :, :])
                wckv = p1w.tile([128, 2, 9, S + 1], F32R, tag="wckv")
                nc.sync.dma_start(out=wckv, in_=wckv_in[:, :, :, :])
                wqkv = p1w.tile([128, 9, 128], F32R, tag="wqkv")
                nc.sync.dma_start(out=wqkv, in_=wqkv_in[:, :, :])
                wf1x = p1w.tile([128, 2, 2, 128], F32R, tag="wf1x")
                nc.sync.dma_start(out=wf1x, in_=wf1x_in[:, :, :, :])
                stat_cq = p1w.tile([S + 1, 1], F32R, tag="stat_cq")
                nc.sync.dma_start(out=stat_cq, in_=stat_cq_in[:, :])
                bc2a = p1w.tile([1, 128], F32R, tag="bc2a")
                nc.sync.dma_start(out=bc2a, in_=bc2a_in[:, :])
                bc2b = p1w.tile([1, 128], F32R, tag="bc2b")
                nc.sync.dma_start(out=bc2b, in_=bc2b_in[:, :])
                bias_q = p1w.tile([S, 1], F32, tag="bias_q")
                nc.sync.dma_start(out=bias_q, in_=bias_q_in[:, :])
                bias_kv = p1w.tile([2 * S, 1], F32, tag="bias_kv")
                nc.sync.dma_start(out=bias_kv, in_=bias_kv_in[:, :])

                xwin = p1w.tile([128, 2, 18, WP], F32R, tag="xwin")
                ywin = p1w.tile([128, 2, 18, WP], F32R, tag="ywin")
                # nwin: xq rows0-63 / ykv rows64-127 ; slot i = row (r0-2)+i ; slot18 always zero
                nwin = p1w.tile([128, 19, WP], F32R, tag="nwin")
                nc.sync.dma_start(out=nwin.rearrange("p a b -> p (a b)"),
                                  in_=zeros_in[:, :19 * WP])

                def q_kv_convs(rq, nrows, sl_base):
                    """q & kv convs for q-rows rq..rq+nrows-1 ; nwin slot of row rq is sl_base."""
                    npx_q = nrows * W
                    ps_q = p1ps.tile([128, NPX], F32, tag="ps_q")
                    ps_kv = p1ps.tile([128, NPX], F32, tag="ps_kv")
                    for t_i, (dy, dx) in enumerate(TAPS):
                        sl0 = sl_base + dy
                        rhs_q = nwin[0:64, sl0:sl0 + nrows, 1 + dx:1 + dx + W]
                        nc.tensor.matmul(ps_q[0:S, 0:npx_q], wqkv[0:64, t_i, 0:S], rhs_q,
                                         start=(t_i == 0), stop=(t_i == 8))
                        rhs_kv = nwin[64:128, sl0:sl0 + nrows, 1 + dx:1 + dx + W]
                        nc.tensor.matmul(ps_kv[:, 0:npx_q], wqkv[64:128, t_i], rhs_kv,
                                         start=(t_i == 0), stop=(t_i == 8))
                    q_acc = p1.tile([S, 1], F32, tag="q_acc")
                    k_acc = p1.tile([S, 1], F32, tag="k_acc")
                    qsb = p1.tile([S, NPX], F32, tag="qsb")
                    qsq = p1.tile([S, NPX], F32, tag="qsq")
                    nc.scalar.activation(qsb[:, 0:npx_q], ps_q[0:S, 0:npx_q], AF.Identity, bias=bias_q)
                    nc.vector.tensor_copy(qk_store[0:S, rq * W:rq * W + npx_q], qsb[:, 0:npx_q])
                    nc.scalar.activation(qsq[:, 0:npx_q], qsb[:, 0:npx_q], AF.Square, accum_out=q_acc)
                    nc.vector.tensor_tensor(rq2, rq2, q_acc, op=ALU.add)
                    kvsb = p1.tile([128, NPX], F32, tag="kvsb")
                    ksq = p1.tile([S, NPX], F32, tag="ksq")
                    nc.scalar.activation(kvsb[:, 0:npx_q], ps_kv[:, 0:npx_q], AF.Identity, bias=bias_kv)
                    nc.vector.tensor_copy(qk_store[64:128, rq * W:rq * W + npx_q], kvsb[0:S, 0:npx_q])
                    nc.vector.tensor_copy(vo_store[0:S, rq * W:rq * W + npx_q], kvsb[S:2 * S, 0:npx_q])
                    nc.scalar.activation(ksq[:, 0:npx_q], kvsb[0:S, 0:npx_q], AF.Square, accum_out=k_acc)
                    nc.vector.tensor_tensor(rk2, rk2, k_acc, op=ALU.add)

                for s_i in range(NSTRIP):
                    r0 = 16 * s_i
                    if s_i > 0:
                        nc.vector.tensor_copy(xwin[:, :, 0:2], xwin[:, :, 16:18])
                        nc.vector.tensor_copy(ywin[:, :, 0:2], ywin[:, :, 16:18])
                        nc.vector.tensor_copy(nwin[:, 0:2], nwin[:, 16:18])
                    for b_i in range(NBLK):
                        rb = r0 + BLK_ROWS * b_i
                        # xwin slot i = row (r0-1)+i  (18 slots: rows r0-1..r0+16)
                        if s_i == 0 and b_i == 0:
                            nc.sync.dma_start(out=xwin[:, :, 0:6], in_=x_in[:, :, 1:7])
                            nc.sync.dma_start(out=ywin[:, :, 0:6], in_=y_in[:, :, 1:7])
                        else:
                            sl = 4 * b_i + 2
                            nc.sync.dma_start(out=xwin[:, :, sl:sl + 4], in_=x_in[:, :, rb + 3:rb + 7])
                            nc.sync.dma_start(out=ywin[:, :, sl:sl + 4], in_=y_in[:, :, rb + 3:rb + 7])
                        # ---- cq / ckv conv + LN -> nwin rows (xq: 0-63, ykv: 64-127)
                        for (name, wl, win_t, dst_lo) in (("cq", wcq, xwin, 0), ("ckv", wckv, ywin, 64)):
                            ps_c = p1ps.tile([128, NPX], F32, tag=f"ps_{name}")
                            first = True
                            for kt in range(2):
                                for t_i, (dy, dx) in enumerate(TAPS):
                                    sl0 = 4 * b_i + 1 + dy
                                    rhs = win_t[:, kt, sl0:sl0 + 4, 1 + dx:1 + dx + W]
                                    nc.tensor.matmul(ps_c[0:S + 1], wl[:, kt, t_i], rhs,
                                                     start=first, stop=(kt == 1 and t_i == 8))
                                    first = False
                            t_sb = p1.tile([S, NPX], F32, tag=f"t_{name}")
                            nc.scalar.copy(t_sb, ps_c[0:S])
                            sq_sb = p1.tile([S + 1, NPX], F32R, tag=f"sq_{name}")
                            nc.scalar.activation(sq_sb, ps_c[0:S + 1], AF.Square)
                            ps_v = p1ps.tile([1, NPX], F32, tag="ps_v")
                            nc.tensor.matmul(ps_v[0:1], stat_cq, sq_sb, start=True, stop=True)
                            mu_t = p1.tile([1, NPX], F32R, tag=f"mut_{name}", bufs=1)
                            nc.scalar.mul(mu_t, ps_c[64:65], 1.0 / S)
                            varr = p1.tile([1, NPX], F32, tag=f"var_{name}", bufs=1)
                            nc.vector.tensor_scalar_add(varr, ps_v[0:1], EPS)
                            rcpv = p1.tile([1, NPX], F32, tag=f"rcp_{name}", bufs=1)
                            nc.vector.reciprocal_approx_fast(out=rcpv, in_=varr)
                            r_t = p1.tile([1, NPX], F32R, tag=f"rt_{name}", bufs=1)
                            nc.scalar.activation(r_t, rcpv, AF.Sqrt)
                            ps_b = p1ps.tile([128, NPX], F32, tag="ps_b")
                            nc.tensor.matmul(ps_b, bc2a, mu_t, start=True, stop=False)
                            nc.tensor.matmul(ps_b, bc2b, r_t, start=False, stop=True)
                            d_sb = p1.tile([S, NPX], F32, tag=f"d_{name}")
                            nc.vector.tensor_tensor(d_sb, t_sb, ps_b[0:S], op=ALU.subtract)
                            dst = nwin[dst_lo:dst_lo + 64, 4 * b_i + 2:4 * b_i + 6, 1:1 + W]
                            nc.vector.tensor_tensor(dst,
                                                    d_sb.rearrange("p (a b) -> p a b", a=4),
                                                    ps_b[64:128].rearrange("p (a b) -> p a b", a=4),
                                                    op=ALU.mult)
                        # ---- fx (ffn1 x-half) rows rb..rb+3
                        for mt in range(2):
                            ps_fx = p1ps.tile([128, NPX], F32, tag="ps_fx")
                            for kt in range(2):
                                rhs = xwin[:, kt, 4 * b_i + 1:4 * b_i + 5, 1:1 + W]
                                nc.tensor.matmul(ps_fx, wf1x[:, kt, mt], rhs,
                                                 start=(kt == 0), stop=(kt == 1))
                            fx_sb = p1.tile([128, NPX], BF16, tag="fx_sb")
                            nc.scalar.copy(fx_sb, ps_fx)
                            nc.sync.dma_start(out=fx_dram[mt, :, rb * W:(rb + 4) * W], in_=fx_sb)
                        # ---- q / kv convs (lag 1 row)
                        if s_i == 0 and b_i == 0:
                            q_kv_convs(0, 3, 2)
                        else:
                            q_kv_convs(rb - 1, 4, 4 * b_i + 1)
                # epilogue: q/kv row 127 (nwin slot of row r = r-110 ; slot18 zero)
                q_kv_convs(127, 1, 17)
                if debug_taps:
                    xq_f = p1.tile([128, HW], F32, tag="xq_f", bufs=1)
                    nc.vector.tensor_copy(xq_f[:, 0:HW], nwin[:, 2:18, 1:1 + W].rearrange("p a b -> p (a b)"))
                    nc.sync.dma_start(out=dbg_xq[:, 14 * 16 * W:HW], in_=xq_f[:, 0:2 * 16 * W])

            # ================= PHASE 2: attention =================
            with tc.tile_pool(name="p2", bufs=2) as p2, \
                 tc.tile_pool(name="p2one", bufs=1) as p2one, \
                 tc.tile_pool(name="p2ps", bufs=2, space="PSUM") as p2ps:
                ident = p2one.tile([128, 128], BF16, tag="ident")
                nc.sync.dma_start(out=ident, in_=ident_in[:, :])
                g_ps = p2ps.tile([S, S], F32, tag="g_ps", bufs=1)
                for tb in range(HW // 128):
                    tp = p2ps.tile([128, 128], BF16, tag="tp")
                    nc.tensor.transpose(tp, qk_store[:, tb * 128:(tb + 1) * 128], ident)
                    tp_sb = p2.tile([128, 128], BF16, tag="tp_sb")
                    nc.scalar.copy(tp_sb, tp)
                    nc.tensor.matmul(g_ps, tp_sb[:, 0:S], tp_sb[:, 64:128],
                                     start=(tb == 0), stop=(tb == HW // 128 - 1))
                g_sb = p2one.tile([S, S], F32, tag="g_sb")
                nc.scalar.copy(g_sb, g_ps)
                rqs = p2one.tile([S, 1], F32, tag="rqs")
                rks = p2one.tile([S, 1], F32, tag="rks")
                sq1 = p2one.tile([S, 1], F32, tag="sq1")
                sq2 = p2one.tile([S, 1], F32, tag="sq2")
                nc.vector.reciprocal_approx_fast(out=sq1, in_=rq2)
                nc.scalar.activation(rqs, sq1, AF.Sqrt)
                nc.vector.reciprocal_approx_fast(out=sq2, in_=rk2)
                nc.scalar.activation(rks, sq2, AF.Sqrt)
                temp_t = p2one.tile([S, 1], F32, tag="temp_t")
                nc.sync.dma_start(out=temp_t, in_=temp_in[:, :])
                nc.vector.tensor_tensor(rqs, rqs, temp_t, op=ALU.mult)
                nc.vector.tensor_scalar_mul(g_sb, g_sb, rqs)
                rk_row = p2one.tile([1, S], F32R, tag="rk_row")
                nc.sync.dma_start(out=rk_row, in_=rks[:, :].bitcast(F32R))
                ones1 = p2one.tile([1, S], F32R, tag="ones1")
                nc.sync.dma_start(out=ones1, in_=ones_in[:, :])
                rkb_ps = p2ps.tile([S, S], F32, tag="rkb_ps", bufs=1)
                nc.tensor.matmul(rkb_ps, ones1, rk_row, start=True, stop=True)
                s_sb = p2one.tile([S, 8, 8], F32, tag="s_sb")
                nc.vector.tensor_tensor(s_sb.rearrange("p a b -> p (a b)"), g_sb, rkb_ps, op=ALU.mult)
                mx = p2one.tile([S, 8], F32, tag="mx")
                nc.vector.reduce_max(mx, s_sb, axis=mybir.AxisListType.X)
                mxb = bass.AP(tensor=mx.tensor, offset=mx.offset,
                              ap=[list(mx.ap[0]), list(mx.ap[1]), [0, 8]])
                e_sb = p2one.tile([S, 8, 8], F32, tag="e_sb")
                nc.vector.tensor_tensor(e_sb, s_sb, mxb, op=ALU.subtract)
                ex_sb = p2one.tile([S, 8, 8], F32, tag="ex_sb")
                nc.scalar.activation(ex_sb, e_sb, AF.Exp)
                sm = p2one.tile([S, 8], F32, tag="sm")
                nc.vector.reduce_sum(sm, ex_sb, axis=mybir.AxisListType.X)
                rs = p2one.tile([S, 8], F32, tag="rs")
                nc.vector.reciprocal_approx_fast(out=rs, in_=sm)
                rsb = bass.AP(tensor=rs.tensor, offset=rs.offset,
                              ap=[list(rs.ap[0]), list(rs.ap[1]), [0, 8]])
                attn = p2one.tile([S, S], F32R, tag="attn")
                nc.vector.tensor_tensor(attn.rearrange("p (a b) -> p a b", a=8), ex_sb, rsb, op=ALU.mult)
                maskbd = p2one.tile([S, S], F32R, tag="maskbd")
                nc.sync.dma_start(out=maskbd, in_=mask_in[:, :])
                attn_m = p2one.tile([S, S], F32R, tag="attn_m")
                nc.vector.tensor_tensor(attn_m, attn, maskbd, op=ALU.mult)
                identr = p2one.tile([S, S], F32R, tag="identr")
                nc.sync.dma_start(out=identr, in_=identr_in[:, :])
                attn_tp = p2ps.tile([S, S], F32R, tag="attn_tp", bufs=1)
                nc.tensor.transpose(attn_tp, attn_m, identr)
                attn_t = p2one.tile([S, S], F32R, tag="attn_t")
                nc.scalar.copy(attn_t, attn_tp)
                if debug_taps:
                    at_f = p2one.tile([S, S + 2], F32, tag="at_f")
                    nc.vector.tensor_copy(at_f[:, 0:S], attn_m[:, :].bitcast(F32))
                    nc.vector.tensor_copy(at_f[:, S:S + 1], rqs)
                    nc.vector.tensor_copy(at_f[:, S + 1:S + 2], rks)
                    nc.sync.dma_start(out=dbg_at[:, :], in_=at_f)
                wpo = p2one.tile([S, S], F32R, tag="wpo")
                nc.sync.dma_start(out=wpo, in_=wpo_in[:, :])
                for blk in range(HW // NPX):
                    vsb = p2.tile([S, NPX], F32R, tag="vsb")
                    nc.vector.tensor_copy(vsb, vo_store[0:S, blk * NPX:(blk + 1) * NPX])
                    ps_o = p2ps.tile([S, NPX], F32, tag="ps_o", bufs=1)
                    nc.tensor.matmul(ps_o, attn_t, vsb, start=True, stop=True)
                    o_sb = p2.tile([S, NPX], F32R, tag="o_sb")
                    nc.scalar.copy(o_sb, ps_o)
                    ps_po = p2ps.tile([S, NPX], F32, tag="ps_po", bufs=1)
                    nc.tensor.matmul(ps_po, wpo, o_sb, start=True, stop=True)
                    nc.vector.tensor_copy(vo_store[64:128, blk * NPX:(blk + 1) * NPX], ps_po)
                if debug_taps:
                    for half in range(2):
                        qk_f = p2.tile([128, HW // 2], F32, tag="qk_f", bufs=1)
                        nc.vector.tensor_copy(qk_f, qk_store[:, half * HW // 2:(half + 1) * HW // 2])
                        nc.sync.dma_start(out=dbg_qk[:, half * HW // 2:(half + 1) * HW // 2], in_=qk_f)
                        vo_f = p2.tile([128, HW // 2], F32, tag="vo_f", bufs=1)
                        nc.vector.tensor_copy(vo_f, vo_store[:, half * HW // 2:(half + 1) * HW // 2])
                        nc.sync.dma_start(out=dbg_vo[:, half * HW // 2:(half + 1) * HW // 2], in_=vo_f)

            # ================= PHASE 3: expand + LN + FFN =================
            with tc.tile_pool(name="p3w", bufs=1) as p3w, \
                 tc.tile_pool(name="p3", bufs=2) as p3, \
                 tc.tile_pool(name="p3ps", bufs=1, space="PSUM") as p3ps:
                wexp = p3w.tile([S, 9, C], BF16, tag="wexp")
                nc.sync.dma_start(out=wexp, in_=wexp_in[:, :, :])
                wf1v = p3w.tile([128, 2, 2, 128], F32R, tag="wf1v")
                nc.sync.dma_start(out=wf1v, in_=wf1v_in[:, :, :, :])
                wdw = p3w.tile([128, 2, 9, 128], F32R, tag="wdw")
                nc.sync.dma_start(out=wdw, in_=wdw_in[:, :, :, :])
                wf2 = p3w.tile([128, 2, 2, 128], F32R, tag="wf2")
                nc.sync.dma_start(out=wf2, in_=wf2_in[:, :, :, :])
                stat256 = p3w.tile([128, 2], F32R, tag="stat256")
                nc.sync.dma_start(out=stat256, in_=stat256_in[:, :])
                ones128 = p3w.tile([1, 128], F32R, tag="ones128")
                nc.sync.dma_start(out=ones128, in_=ones128_in[:, :])
                bias_g = p3w.tile([128, 2, 1], F32, tag="bias_g")
                nc.sync.dma_start(out=bias_g, in_=bias_g_in[:, :, :])
                # owin: slot i = o row (r0-2)+i ; slot18 zero
                owin = p3w.tile([S, 19, WP], BF16, tag="owin")
                nc.sync.dma_start(out=owin.rearrange("p a b -> p (a b)"), in_=zerosb_in[0:S, :19 * WP])
                # f1win: slot i = f1 row (r0-3)+i (slots 0..18); slot19 always zero
                f1win = p3w.tile([128, 2, 20, WP], F32R, tag="f1win")
                for half in range(2):
                    nc.sync.dma_start(out=f1win[:, half].rearrange("p a b -> p (a b)"),
                                      in_=zeros_in[:, :20 * WP])

                def stage_a(re, nrows, slo, b_i, s_i):
                    """expand conv rows re..re+nrows-1 (owin slot of row re = slo) + LN + ffn1 -> f1win"""
                    npx_e = nrows * W
                    ps_e0 = p3ps.tile([128, NPX], F32, tag="ps_e0")
                    ps_e1 = p3ps.tile([128, NPX], F32, tag="ps_e1")
                    for t_i, (dy, dx) in enumerate(TAPS):
                        sl0 = slo + dy
                        rhs = owin[:, sl0:sl0 + nrows, 1 + dx:1 + dx + W]
                        nc.tensor.matmul(ps_e0[:, 0:npx_e], wexp[:, t_i, 0:128], rhs,
                                         start=(t_i == 0), stop=(t_i == 8))
                        nc.tensor.matmul(ps_e1[:, 0:npx_e], wexp[:, t_i, 128:256], rhs,
                                         start=(t_i == 0), stop=(t_i == 8))
                    t0 = p3.tile([128, NPX], F32R, tag="t0")
                    t1 = p3.tile([128, NPX], F32R, tag="t1")
                    nc.scalar.copy(t0[:, 0:npx_e], ps_e0[:, 0:npx_e])
                    nc.scalar.copy(t1[:, 0:npx_e], ps_e1[:, 0:npx_e])
                    sq0 = p3.tile([128, NPX], F32R, tag="sq0")
                    sq1t = p3.tile([128, NPX], F32R, tag="sq1t")
                    nc.scalar.activation(sq0[:, 0:npx_e], ps_e0[:, 0:npx_e], AF.Square)
                    nc.scalar.activation(sq1t[:, 0:npx_e], ps_e1[:, 0:npx_e], AF.Square)
                    ps_stm = p3ps.tile([1, NPX], F32, tag="small", bufs=2)
                    nc.tensor.matmul(ps_stm[0:1, 0:npx_e], stat256[:, 0:1], t0[:, 0:npx_e], start=True, stop=False)
                    nc.tensor.matmul(ps_stm[0:1, 0:npx_e], stat256[:, 0:1], t1[:, 0:npx_e], start=False, stop=True)
                    ps_sts = p3ps.tile([1, NPX], F32, tag="small", bufs=2)
                    nc.tensor.matmul(ps_sts[0:1, 0:npx_e], stat256[:, 1:2], sq0[:, 0:npx_e], start=True, stop=False)
                    nc.tensor.matmul(ps_sts[0:1, 0:npx_e], stat256[:, 1:2], sq1t[:, 0:npx_e], start=False, stop=True)
                    mu3 = p3.tile([1, NPX], F32R, tag="mu3", bufs=1)
                    nc.scalar.copy(mu3[:, 0:npx_e], ps_stm[0:1, 0:npx_e])
                    musq = p3.tile([1, NPX], F32, tag="musq", bufs=1)
                    mu3v = mu3[:, 0:npx_e].bitcast(F32)
                    nc.vector.tensor_tensor(musq[:, 0:npx_e], mu3v, mu3v, op=ALU.mult)
                    varr = p3.tile([1, NPX], F32, tag="varr", bufs=1)
                    nc.vector.scalar_tensor_tensor(varr[:, 0:npx_e], ps_sts[0:1, 0:npx_e], EPS,
                                                   musq[:, 0:npx_e], op0=ALU.add, op1=ALU.subtract)
                    rcpv = p3.tile([1, NPX], F32, tag="rcpv", bufs=1)
                    nc.vector.reciprocal_approx_fast(out=rcpv[:, 0:npx_e], in_=varr[:, 0:npx_e])
                    r3 = p3.tile([1, NPX], F32R, tag="r3", bufs=1)
                    nc.scalar.activation(r3[:, 0:npx_e], rcpv[:, 0:npx_e], AF.Sqrt)
                    ps_mu = p3ps.tile([128, NPX], F32, tag="small", bufs=2)
                    nc.tensor.matmul(ps_mu[:, 0:npx_e], ones128, mu3[:, 0:npx_e], start=True, stop=True)
                    ps_r = p3ps.tile([128, NPX], F32, tag="small", bufs=2)
                    nc.tensor.matmul(ps_r[:, 0:npx_e], ones128, r3[:, 0:npx_e], start=True, stop=True)
                    vn0 = p3.tile([128, NPX], F32R, tag="vn0")
                    vn1 = p3.tile([128, NPX], F32R, tag="vn1")
                    for vt, tt in ((vn0, t0), (vn1, t1)):
                        dsb = p3.tile([128, NPX], F32, tag="dsb")
                        nc.vector.tensor_tensor(dsb[:, 0:npx_e], tt[:, 0:npx_e], ps_mu[:, 0:npx_e], op=ALU.subtract)
                        nc.vector.tensor_tensor(vt[:, 0:npx_e], dsb[:, 0:npx_e], ps_r[:, 0:npx_e], op=ALU.mult)
                    # ffn1-v + fx -> f1win rows re.. (slot = re-(r0-3) = slo+1)
                    for mt in range(2):
                        ps_f = p3ps.tile([128, NPX], F32, tag="ps_f")
                        nc.tensor.matmul(ps_f[:, 0:npx_e], wf1v[:, 0, mt], vn0[:, 0:npx_e], start=True, stop=False)
                        nc.tensor.matmul(ps_f[:, 0:npx_e], wf1v[:, 1, mt], vn1[:, 0:npx_e], start=False, stop=True)
                        fxs = p3.tile([128, NPX], BF16, tag="fxs")
                        nc.sync.dma_start(out=fxs[:, 0:npx_e], in_=fx_dram[mt, :, re * W:re * W + npx_e])
                        f1t = p3.tile([128, NPX], F32R, tag="f1t")
                        nc.vector.tensor_tensor(f1t[:, 0:npx_e], ps_f[:, 0:npx_e], fxs[:, 0:npx_e], op=ALU.add)
                        dstf = f1win[:, mt, slo + 1:slo + 1 + nrows, 1:1 + W]
                        nc.vector.tensor_copy(dstf,
                                              f1t[:, 0:npx_e].rearrange("p (a b) -> p a b", a=nrows))
                        if debug_taps:
                            f1c = p3.tile([128, NPX], F32, tag="f1c")
                            nc.vector.tensor_copy(f1c[:, 0:npx_e], f1t[:, 0:npx_e])
                            nc.sync.dma_start(out=dbg_f1[:, mt, re * W:re * W + npx_e], in_=f1c[:, 0:npx_e])

                def stage_b(rg, nrg, slg):
                    """dw conv rows rg..rg+nrg-1 (f1win slot of row rg = slg) + gelu + ffn2 -> out"""
                    npx_g = nrg * W
                    gsb = p3.tile([128, 2, NPX], F32R, tag="gsb")
                    for ct in range(2):
                        ps_g = p3ps.tile([128, NPX], F32, tag="ps_g")
                        for t_i, (dy, dx) in enumerate(TAPS):
                            sl0 = slg + dy
                            rhs = f1win[:, ct, sl0:sl0 + nrg, 1 + dx:1 + dx + W]
                            nc.tensor.matmul(ps_g[:, 0:npx_g], wdw[:, ct, t_i], rhs,
                                             start=(t_i == 0), stop=(t_i == 8))
                        nc.scalar.activation(gsb[:, ct, 0:npx_g], ps_g[:, 0:npx_g], AF.Gelu,
                                             bias=bias_g[:, ct])
                    for mt in range(2):
                        ps_out = p3ps.tile([128, NPX], F32, tag="ps_out")
                        nc.tensor.matmul(ps_out[:, 0:npx_g], wf2[:, 0, mt], gsb[:, 0, 0:npx_g], start=True, stop=False)
                        nc.tensor.matmul(ps_out[:, 0:npx_g], wf2[:, 1, mt], gsb[:, 1, 0:npx_g], start=False, stop=True)
                        osb = p3.tile([128, NPX], F32, tag="osb")
                        nc.scalar.copy(osb[:, 0:npx_g], ps_out[:, 0:npx_g])
                        nc.sync.dma_start(out=out_dram[mt, :, rg * W:rg * W + npx_g], in_=osb[:, 0:npx_g])

                for s_i in range(NSTRIP):
                    r0 = 16 * s_i
                    if s_i > 0:
                        nc.vector.tensor_copy(owin[:, 0:2], owin[:, 16:18])
                        nc.vector.tensor_copy(f1win[:, :, 0:3], f1win[:, :, 16:19])
                    # stage A over blocks
                    for b_i in range(NBLK):
                        rb = r0 + BLK_ROWS * b_i
                        dsto = owin[:, 4 * b_i + 2:4 * b_i + 6, 1:1 + W]
                        nc.vector.tensor_copy(dsto,
                                              vo_store[64:128, rb * W:(rb + 4) * W].rearrange("p (a b) -> p a b", a=4))
                        if s_i == 0 and b_i == 0:
                            stage_a(0, 3, 2, b_i, s_i)
                        else:
                            stage_a(rb - 1, 4, 4 * b_i + 1, b_i, s_i)
                    if s_i == NSTRIP - 1:
                        # f1 row 127 epilogue (o rows 126..128 ; owin slot of row 126 = 16)
                        stage_a(127, 1, 17, 0, s_i)
                    # stage B over blocks (rows r0-2 .. r0+13)
                    for b_i in range(NBLK):
                        rb = r0 + BLK_ROWS * b_i
                        if s_i == 0 and b_i == 0:
                            stage_b(0, 2, 3)
                        else:
                            stage_b(rb - 2, 4, 4 * b_i + 1)
                # out rows 126,127 (f1win slot of row 126 = 126-109 = 17 ; slot19 zero? need row 128->slot19)
                stage_b(126, 2, 17)
    return nc


def _prep_host(inputs):
    f32 = np.float32
    w_cq = np.asarray(inputs["w_cq"], f32)
    w_ckv = np.asarray(inputs["w_ckv"], f32)
    ln_q_w = np.asarray(inputs["ln_q_w"], f32); ln_q_b = np.asarray(inputs["ln_q_b"], f32)
    ln_kv_w = np.asarray(inputs["ln_kv_w"], f32); ln_kv_b = np.asarray(inputs["ln_kv_b"], f32)
    w_kv = np.asarray(inputs["w_kv"], f32)
    w_kvdw = np.asarray(inputs["w_kvdw"], f32)
    w_q = np.asarray(inputs["w_q"], f32)
    temperature = np.asarray(inputs["temperature"], f32)
    w_po = np.asarray(inputs["w_po"], f32)
    w_expand = np.asarray(inputs["w_expand"], f32)
    ln_out_w = np.asarray(inputs["ln_out_w"], f32); ln_out_b = np.asarray(inputs["ln_out_b"], f32)
    w_ffn1 = np.asarray(inputs["w_ffn1"], f32)
    w_ffn_dw = np.asarray(inputs["w_ffn_dw"], f32)
    w_ffn2 = np.asarray(inputs["w_ffn2"], f32)

    d = {}
    def conv_lhsT(wc):
        a = np.zeros((128, 2, 9, S + 1), f32)
        for kt in range(2):
            blk = wc[:, kt * 128:(kt + 1) * 128]           # [S, 128, 3, 3]
            a[:, kt, :, :S] = blk.transpose(1, 2, 3, 0).reshape(128, 9, S)
            a[:, kt, :, S] = blk.sum(axis=0).reshape(128, 9)
        return a
    d["wcq"] = conv_lhsT(w_cq)
    d["wckv"] = conv_lhsT(w_ckv)
    w_q_eff = w_q * ln_q_w[None, :, None, None]
    d["bias_q"] = (w_q * ln_q_b[None, :, None, None]).sum(axis=(1, 2, 3)).reshape(S, 1)
    wqkv = np.zeros((128, 9, 128), f32)
    wqkv[0:64, :, 0:S] = w_q_eff.transpose(1, 2, 3, 0).reshape(S, 9, S)
    w_kv_g = w_kv[:, :, 0, 0] * ln_kv_w[None, :]
    w_kv_eff = w_kvdw[:, 0][:, None] * w_kv_g[:, :, None, None]   # [2S, S, 3, 3]
    d["bias_kv"] = (w_kvdw[:, 0].sum(axis=(1, 2)) * (w_kv[:, :, 0, 0] @ ln_kv_b)).reshape(2 * S, 1)
    wqkv[64:128, :, :] = w_kv_eff.transpose(1, 2, 3, 0).reshape(S, 9, 2 * S)
    d["wqkv"] = wqkv
    d["wpo"] = np.ascontiguousarray(w_po[:, :, 0, 0].T)
    d["wexp"] = np.ascontiguousarray(
        w_expand.transpose(1, 2, 3, 0).reshape(S, 9, C)).astype(ml_dtypes.bfloat16)
    w1 = w_ffn1[:, :, 0, 0]
    w1x = w1[:, :C]
    w1v = w1[:, C:] * ln_out_w[None, :]
    def one_by_one_lhsT(wm):
        a = np.zeros((128, 2, 2, 128), f32)
        for kt in range(2):
            for mt in range(2):
                a[:, kt, mt, :] = wm[mt * 128:(mt + 1) * 128, kt * 128:(kt + 1) * 128].T
        return a
    d["wf1x"] = one_by_one_lhsT(w1x)
    d["wf1v"] = one_by_one_lhsT(w1v)
    bias_f1 = w1[:, C:] @ ln_out_b
    dw_t = w_ffn_dw[:, 0].reshape(C, 9)
    d["bias_g"] = np.ascontiguousarray(
        (bias_f1 * dw_t.sum(1)).reshape(2, 128, 1).transpose(1, 0, 2))
    wdw = np.zeros((128, 2, 9, 128), f32)
    for ct in range(2):
        for t in range(9):
            np.fill_diagonal(wdw[:, ct, t, :], dw_t[ct * 128:(ct + 1) * 128, t])
    d["wdw"] = wdw
    d["wf2"] = one_by_one_lhsT(w_ffn2[:, :, 0, 0])
    stat_cq = np.zeros((S + 1, 1), f32)
    stat_cq[:S, 0] = 1.0 / S
    stat_cq[S, 0] = -1.0 / (S * S)
    d["stat_cq"] = stat_cq
    bc2a = np.zeros((1, 128), f32); bc2a[0, 0:64] = 1.0
    bc2b = np.zeros((1, 128), f32); bc2b[0, 64:128] = 1.0
    d["bc2a"] = bc2a; d["bc2b"] = bc2b
    d["ones128"] = np.ones((1, 128), f32)
    stat256 = np.zeros((128, 2), f32)
    stat256[:, 0] = 1.0 / C
    stat256[:, 1] = 1.0 / C
    d["stat256"] = stat256
    d["tempv"] = np.repeat(temperature.reshape(HEADS), S // HEADS).reshape(S, 1).astype(f32)
    mask = np.zeros((S, S), f32)
    for h in range(HEADS):
        mask[h * 8:(h + 1) * 8, h * 8:(h + 1) * 8] = 1.0
    d["maskbd"] = mask
    d["ident"] = np.eye(128, dtype=f32).astype(ml_dtypes.bfloat16)
    d["identr"] = np.eye(S, dtype=f32)
    d["onesr"] = np.ones((1, S), f32)
    d["zeros"] = np.zeros((128, 2600), f32)
    d["zerosb"] = np.zeros((128, 2600), f32).astype(ml_dtypes.bfloat16)
    return d


def _pad_input(x):
    """[C,H,W] f32 -> [128, 2, H+4, WP] zero-padded, partition-major"""
    out = np.zeros((128, 2, H + 4, WP), np.float32)
    out[:, :, 2:H + 2, 1:W + 1] = x.reshape(2, 128, H, W).transpose(1, 0, 2, 3)
    return out


def kernel(**inputs):
    key = "nc"
    if key not in _CACHED:
        nc = build_nc(debug_taps=False)
        nc.finalize()
        _CACHED[key] = nc
    nc = _CACHED[key]
    d = _prep_host(inputs)
    x = np.asarray(inputs["x"], np.float32)
    y = np.asarray(inputs["y"], np.float32)
    in_maps = []
    for i in range(B):
        m = dict(d)
        m["x"] = _pad_input(x[i])
        m["y"] = _pad_input(y[i])
        in_maps.append(m)
    res = run_bass_kernel_spmd(nc, in_maps, list(range(B)))
    out = np.stack([res.results[i]["out"].reshape(C, H, W) for i in range(B)])
    return out.astype(np.float32)



# revision 44
# speedup vs baseline: 101.9370x; 101.9370x over previous
# Trainium2 Bass kernel for ChannelAttentionBlock (B=8,C=256,H=W=128,S=64,HEADS=8)
# Data-parallel over batch: 1 sample per NeuronCore, 8 cores.
import numpy as np
import ml_dtypes

import concourse.bass as bass
from concourse import bacc
import concourse.mybir as mybir
from concourse.bass_utils import run_bass_kernel_spmd
from concourse.tile import TileContext

F32R = mybir.dt.float32r
F32 = mybir.dt.float32
BF16 = mybir.dt.bfloat16
AF = mybir.ActivationFunctionType
ALU = mybir.AluOpType

B, C, H, W = 8, 256, 128, 128
S = 64
HEADS = 8
HW = H * W
WP = W + 2          # padded row width
R = 16              # rows per strip
NSTRIP = H // R
BLK_ROWS = 4
NBLK = 4
NPX = BLK_ROWS * W  # 512
EPS = 1e-5

TAPS = [(dy, dx) for dy in (-1, 0, 1) for dx in (-1, 0, 1)]

_CACHED = {}


def build_nc(debug_taps=False):
    nc = bacc.Bacc("TRN2", target_bir_lowering=False, debug=False)

    # ------------- DRAM parameters (host layouts) -------------
    # x/y: [128(part), 2(ktile), H+4(rows: img rows -2..129), WP]
    x_in = nc.dram_tensor("x", [128, 2, H + 4, WP], BF16, kind="ExternalInput")
    y_in = nc.dram_tensor("y", [128, 2, H + 4, WP], BF16, kind="ExternalInput")
    wcq_in = nc.dram_tensor("wcq", [128, 2, 9, S + 1], BF16, kind="ExternalInput")
    wckv_in = nc.dram_tensor("wckv", [128, 2, 9, S + 1], BF16, kind="ExternalInput")
    wqkv_in = nc.dram_tensor("wqkv", [128, 9, 128], BF16, kind="ExternalInput")
    wpo_in = nc.dram_tensor("wpo", [S, S], BF16, kind="ExternalInput")
    wexpa_in = nc.dram_tensor("wexpa", [128, 3, C], BF16, kind="ExternalInput")
    wexps_in = nc.dram_tensor("wexps", [S, 3, C], BF16, kind="ExternalInput")
    wf1x_in = nc.dram_tensor("wf1x", [128, 2, 2, 128], BF16, kind="ExternalInput")
    wf1v_in = nc.dram_tensor("wf1v", [128, 2, 2, 128], BF16, kind="ExternalInput")
    wdw_in = nc.dram_tensor("wdw", [128, 2, 9, 128], BF16, kind="ExternalInput")
    wf2_in = nc.dram_tensor("wf2", [128, 2, 2, 128], BF16, kind="ExternalInput")
    stat_cq_in = nc.dram_tensor("stat_cq", [S + 1, 1], BF16, kind="ExternalInput")
    bc2a_in = nc.dram_tensor("bc2a", [1, 128], BF16, kind="ExternalInput")    # 1s at rows0-63
    bc2b_in = nc.dram_tensor("bc2b", [1, 128], BF16, kind="ExternalInput")    # 1s at rows64-127
    ones128_in = nc.dram_tensor("ones128", [1, 128], BF16, kind="ExternalInput")
    stat256_in = nc.dram_tensor("stat256", [128, 2], BF16, kind="ExternalInput")
    bias_q_in = nc.dram_tensor("bias_q", [S, 1], F32, kind="ExternalInput")
    bias_kv_in = nc.dram_tensor("bias_kv", [2 * S, 1], F32, kind="ExternalInput")
    bias_g_in = nc.dram_tensor("bias_g", [128, 2, 1], F32, kind="ExternalInput")
    temp_in = nc.dram_tensor("tempv", [S, 1], F32, kind="ExternalInput")
    mask_in = nc.dram_tensor("maskbd", [S, S], F32R, kind="ExternalInput")
    ident_in = nc.dram_tensor("ident", [128, 128], BF16, kind="ExternalInput")
    identr_in = nc.dram_tensor("identr", [S, S], F32R, kind="ExternalInput")
    ones_in = nc.dram_tensor("onesr", [1, S], F32R, kind="ExternalInput")
    zerosb_in = nc.dram_tensor("zerosb", [128, 3000], BF16, kind="ExternalInput")

    fx_dram = nc.dram_tensor("fx_dram", [2, 128, HW], BF16)
    out_dram = nc.dram_tensor("out", [2, 128, HW], BF16, kind="ExternalOutput")
    if debug_taps:
        dbg_xq = nc.dram_tensor("dbg_xq", [128, HW], F32, kind="ExternalOutput")
        dbg_qk = nc.dram_tensor("dbg_qk", [128, HW], F32, kind="ExternalOutput")
        dbg_vo = nc.dram_tensor("dbg_vo", [128, HW], F32, kind="ExternalOutput")
        dbg_at = nc.dram_tensor("dbg_at", [S, S + 2], F32, kind="ExternalOutput")
        dbg_f1 = nc.dram_tensor("dbg_f1", [128, 2, HW], F32, kind="ExternalOutput")

    with TileContext(nc) as tc:
        with tc.tile_pool(name="persist", bufs=1) as persist, \
             tc.tile_pool(name="psump", bufs=1, space="PSUM") as psump:
            qk_store = persist.tile([128, HW], BF16, tag="qk_store")
            vo_store = persist.tile([128, HW], BF16, tag="vo_store")
            rq2 = persist.tile([S, 1], F32, tag="rq2")
            rk2 = persist.tile([S, 1], F32, tag="rk2")
            g_ps = psump.tile([S, S], F32, tag="g_ps")
            ident = persist.tile([128, 128], BF16, tag="ident")
            nc.sync.dma_start(out=ident, in_=ident_in[:, :])
            nc.vector.memset(rq2, 0.0)
            nc.vector.memset(rk2, 0.0)

            # ================= PHASE 1 =================
            with tc.tile_pool(name="p1w", bufs=1) as p1w, \
                 tc.tile_pool(name="p1", bufs=2) as p1, \
                 tc.tile_pool(name="p1ps", bufs=1, space="PSUM") as p1ps:
                # first-needed data first: block-0 windows + cq/ckv weights
                xwin = p1w.tile([128, 2, 18, WP], BF16, tag="xwin")
                ywin = p1w.tile([128, 2, 18, WP], BF16, tag="ywin")
                wcq = p1w.tile([128, 2, 9, S + 1], BF16, tag="wcq")
                nc.sync.dma_start(out=wcq, in_=wcq_in[:, :, :, :])
                nc.sync.dma_start(out=xwin[:, :, 0:6], in_=x_in[:, :, 1:7])
                wckv = p1w.tile([128, 2, 9, S + 1], BF16, tag="wckv")
                nc.sync.dma_start(out=wckv, in_=wckv_in[:, :, :, :])
                nc.sync.dma_start(out=ywin[:, :, 0:6], in_=y_in[:, :, 1:7])
                wqkv = p1w.tile([128, 9, 128], BF16, tag="wqkv")
                nc.sync.dma_start(out=wqkv, in_=wqkv_in[:, :, :])
                wf1x = p1w.tile([128, 2, 2, 128], BF16, tag="wf1x")
                nc.sync.dma_start(out=wf1x, in_=wf1x_in[:, :, :, :])
                stat_cq = p1w.tile([S + 1, 1], BF16, tag="stat_cq")
                nc.sync.dma_start(out=stat_cq, in_=stat_cq_in[:, :])
                bc2a = p1w.tile([1, 128], BF16, tag="bc2a")
                nc.sync.dma_start(out=bc2a, in_=bc2a_in[:, :])
                bc2b = p1w.tile([1, 128], BF16, tag="bc2b")
                nc.sync.dma_start(out=bc2b, in_=bc2b_in[:, :])
                bias_q = p1w.tile([S, 1], F32, tag="bias_q")
                nc.sync.dma_start(out=bias_q, in_=bias_q_in[:, :])
                bias_kv = p1w.tile([2 * S, 1], F32, tag="bias_kv")
                nc.sync.dma_start(out=bias_kv, in_=bias_kv_in[:, :])

                # nwin: xq rows0-63 / ykv rows64-127 ; slot i = row (r0-2)+i ; slot18 always zero
                nwin = p1w.tile([128, 19, WP], BF16, tag="nwin")
                nc.sync.dma_start(out=nwin.rearrange("p a b -> p (a b)"),
                                  in_=zerosb_in[:, :19 * WP])

                def conv_mms(name, wl, win_t, b_i):
                    ps_c = p1ps.tile([128, NPX], F32, tag=f"ps_{name}")
                    first = True
                    for kt in range(2):
                        for t_i, (dy, dx) in enumerate(TAPS):
                            sl0 = 4 * b_i + 1 + dy
                            rhs = win_t[:, kt, sl0:sl0 + 4, 1 + dx:1 + dx + W]
                            nc.tensor.matmul(ps_c[0:S + 1], wl[:, kt, t_i], rhs,
                                             start=first, stop=(kt == 1 and t_i == 8))
                            first = False
                    return ps_c

                def ln_stats(name, ps_c):
                    sq_sb = p1.tile([S + 1, NPX], BF16, tag=f"sq_{name}")
                    nc.scalar.activation(sq_sb, ps_c[0:S + 1], AF.Square)
                    t_sb = p1.tile([S, NPX], BF16, tag=f"t_{name}")
                    nc.scalar.copy(t_sb, ps_c[0:S])
                    mu_t = p1.tile([1, NPX], BF16, tag=f"mut_{name}", bufs=1)
                    nc.scalar.mul(mu_t, ps_c[64:65], 1.0 / S)
                    ps_v = p1ps.tile([1, NPX], F32, tag="ps_v")
                    nc.tensor.matmul(ps_v[0:1], stat_cq, sq_sb, start=True, stop=True)
                    return t_sb, mu_t, ps_v

                def ln_mid(name, ps_v):
                    varr = p1.tile([1, NPX], F32, tag=f"var_{name}", bufs=1)
                    nc.vector.tensor_scalar_add(varr, ps_v[0:1], EPS)
                    rcpv = p1.tile([1, NPX], F32, tag=f"rcp_{name}", bufs=1)
                    nc.vector.reciprocal_approx_fast(out=rcpv, in_=varr)
                    r_t = p1.tile([1, NPX], BF16, tag=f"rt_{name}", bufs=1)
                    nc.scalar.activation(r_t, rcpv, AF.Sqrt)
                    return r_t

                def ln_apply(name, t_sb, mu_t, r_t, dst_lo, b_i):
                    ps_b = p1ps.tile([128, NPX], F32, tag="ps_b")
                    nc.tensor.matmul(ps_b, bc2a, mu_t, start=True, stop=False)
                    nc.tensor.matmul(ps_b, bc2b, r_t, start=False, stop=True)
                    d_sb = p1.tile([S, NPX], F32, tag=f"d_{name}")
                    nc.vector.tensor_tensor(d_sb, t_sb, ps_b[0:S], op=ALU.subtract)
                    dst = nwin[dst_lo:dst_lo + 64, 4 * b_i + 2:4 * b_i + 6, 1:1 + W]
                    nc.vector.tensor_tensor(dst,
                                            d_sb.rearrange("p (a b) -> p a b", a=4),
                                            ps_b[64:128].rearrange("p (a b) -> p a b", a=4),
                                            op=ALU.mult)

                def q_kv_convs(rq, nrows, sl_base):
                    """q & kv convs for q-rows rq..rq+nrows-1 ; nwin slot of row rq is sl_base."""
                    npx_q = nrows * W
                    ps_q = p1ps.tile([128, NPX], F32, tag="ps_q")
                    ps_kv = p1ps.tile([128, NPX], F32, tag="ps_kv")
                    for t_i, (dy, dx) in enumerate(TAPS):
                        sl0 = sl_base + dy
                        rhs_q = nwin[0:64, sl0:sl0 + nrows, 1 + dx:1 + dx + W]
                        nc.tensor.matmul(ps_q[0:S, 0:npx_q], wqkv[0:64, t_i, 0:S], rhs_q,
                                         start=(t_i == 0), stop=(t_i == 8))
                        rhs_kv = nwin[64:128, sl0:sl0 + nrows, 1 + dx:1 + dx + W]
                        nc.tensor.matmul(ps_kv[:, 0:npx_q], wqkv[64:128, t_i], rhs_kv,
                                         start=(t_i == 0), stop=(t_i == 8))
                    q_acc = p1.tile([S, 1], F32, tag="q_acc")
                    k_acc = p1.tile([S, 1], F32, tag="k_acc")
                    qsq = p1.tile([S, NPX], BF16, tag="qsq")
                    ksq = p1.tile([S, NPX], BF16, tag="ksq")
                    nc.scalar.activation(qk_store[0:S, rq * W:rq * W + npx_q],
                                         ps_q[0:S, 0:npx_q], AF.Identity, bias=bias_q)
                    nc.scalar.activation(qsq[:, 0:npx_q], ps_q[0:S, 0:npx_q], AF.Square,
                                         bias=bias_q, accum_out=q_acc)
                    nc.vector.tensor_tensor(rq2, rq2, q_acc, op=ALU.add)
                    nc.scalar.activation(qk_store[64:128, rq * W:rq * W + npx_q],
                                         ps_kv[0:S, 0:npx_q], AF.Identity, bias=bias_kv[0:S])
                    nc.scalar.activation(ksq[:, 0:npx_q], ps_kv[0:S, 0:npx_q], AF.Square,
                                         bias=bias_kv[0:S], accum_out=k_acc)
                    nc.vector.tensor_tensor(rk2, rk2, k_acc, op=ALU.add)
                    nc.scalar.activation(vo_store[0:S, rq * W:rq * W + npx_q],
                                         ps_kv[S:2 * S, 0:npx_q], AF.Identity, bias=bias_kv[S:2 * S])

                for s_i in range(NSTRIP):
                    r0 = 16 * s_i
                    if s_i > 0:
                        nc.vector.tensor_copy(xwin[:, :, 0:2], xwin[:, :, 16:18])
                        nc.vector.tensor_copy(ywin[:, :, 0:2], ywin[:, :, 16:18])
                        nc.vector.tensor_copy(nwin[:, 0:2], nwin[:, 16:18])
                    for b_i in range(NBLK):
                        g = 4 * s_i + b_i
                        rb = r0 + BLK_ROWS * b_i
                        # xwin slot i = row (r0-1)+i  (18 slots: rows r0-1..r0+16)
                        if s_i == 0 and b_i == 0:
                            pass  # block-0 windows prefetched at pool start
                        else:
                            sl = 4 * b_i + 2
                            nc.sync.dma_start(out=xwin[:, :, sl:sl + 4], in_=x_in[:, :, rb + 3:rb + 7])
                            nc.sync.dma_start(out=ywin[:, :, sl:sl + 4], in_=y_in[:, :, rb + 3:rb + 7])
                        # cq / ckv conv + LN -> nwin
                        for (name, wl, win_t, dst_lo) in (("cq", wcq, xwin, 0), ("ckv", wckv, ywin, 64)):
                            ps_c = conv_mms(name, wl, win_t, b_i)
                            t_c, mu_c, psv_c = ln_stats(name, ps_c)
                            r_c = ln_mid(name, psv_c)
                            ln_apply(name, t_c, mu_c, r_c, dst_lo, b_i)
                        # fx (ffn1 x-half) rows rb..rb+3
                        for mt in range(2):
                            ps_fx = p1ps.tile([128, NPX], F32, tag="ps_fx")
                            for kt in range(2):
                                rhs = xwin[:, kt, 4 * b_i + 1:4 * b_i + 5, 1:1 + W]
                                nc.tensor.matmul(ps_fx, wf1x[:, kt, mt], rhs,
                                                 start=(kt == 0), stop=(kt == 1))
                            fx_sb = p1.tile([128, NPX], BF16, tag="fx_sb")
                            nc.scalar.copy(fx_sb, ps_fx)
                            nc.sync.dma_start(out=fx_dram[mt, :, rb * W:(rb + 4) * W], in_=fx_sb)
                        # q / kv convs (lag 1 row)
                        if s_i == 0 and b_i == 0:
                            q_kv_convs(0, 3, 2)
                        else:
                            q_kv_convs(rb - 1, 4, 4 * b_i + 1)
                # epilogue: q/kv row 127 (nwin slot of row r = r-110 ; slot18 zero)
                q_kv_convs(127, 1, 17)
                if debug_taps:
                    xq_f = p1.tile([128, HW], F32, tag="xq_f", bufs=1)
                    nc.vector.tensor_copy(xq_f[:, 0:HW], nwin[:, 2:18, 1:1 + W].rearrange("p a b -> p (a b)"))
                    nc.sync.dma_start(out=dbg_xq[:, 14 * 16 * W:HW], in_=xq_f[:, 0:2 * 16 * W])

            # ================= PHASE 2: attention =================
            with tc.tile_pool(name="p2", bufs=2) as p2, \
                 tc.tile_pool(name="p2one", bufs=1) as p2one, \
                 tc.tile_pool(name="p2ps", bufs=2, space="PSUM") as p2ps:
                for tb in range(HW // 512):
                    tp4 = p2ps.tile([128, 4, 128], BF16, tag="tp4")
                    for j in range(4):
                        nc.tensor.transpose(tp4[:, j],
                                            qk_store[:, (4 * tb + j) * 128:(4 * tb + j + 1) * 128], ident)
                    tp_sb = p2.tile([128, 4, 128], BF16, tag="tp_sb")
                    nc.scalar.copy(tp_sb, tp4)
                    for j in range(4):
                        nc.tensor.matmul(g_ps, tp_sb[:, j, 0:S], tp_sb[:, j, 64:128],
                                         start=(tb == 0 and j == 0),
                                         stop=(tb == HW // 512 - 1 and j == 3))
                g_sb = p2one.tile([S, S], F32, tag="g_sb")
                nc.scalar.copy(g_sb, g_ps)
                rqs = p2one.tile([S, 1], F32, tag="rqs")
                rks = p2one.tile([S, 1], F32, tag="rks")
                sq1 = p2one.tile([S, 1], F32, tag="sq1")
                sq2 = p2one.tile([S, 1], F32, tag="sq2")
                nc.vector.reciprocal_approx_fast(out=sq1, in_=rq2)
                nc.scalar.activation(rqs, sq1, AF.Sqrt)
                nc.vector.reciprocal_approx_fast(out=sq2, in_=rk2)
                nc.scalar.activation(rks, sq2, AF.Sqrt)
                temp_t = p2one.tile([S, 1], F32, tag="temp_t")
                nc.sync.dma_start(out=temp_t, in_=temp_in[:, :])
                nc.vector.tensor_tensor(rqs, rqs, temp_t, op=ALU.mult)
                nc.vector.tensor_scalar_mul(g_sb, g_sb, rqs)
                rk_row = p2one.tile([1, S], F32R, tag="rk_row")
                nc.sync.dma_start(out=rk_row, in_=rks[:, :].bitcast(F32R))
                ones1 = p2one.tile([1, S], F32R, tag="ones1")
                nc.sync.dma_start(out=ones1, in_=ones_in[:, :])
                rkb_ps = p2ps.tile([S, S], F32, tag="rkb_ps", bufs=1)
                nc.tensor.matmul(rkb_ps, ones1, rk_row, start=True, stop=True)
                s_sb = p2one.tile([S, 8, 8], F32, tag="s_sb")
                nc.vector.tensor_tensor(s_sb.rearrange("p a b -> p (a b)"), g_sb, rkb_ps, op=ALU.mult)
                mx = p2one.tile([S, 8], F32, tag="mx")
                nc.vector.reduce_max(mx, s_sb, axis=mybir.AxisListType.X)
                mxb = bass.AP(tensor=mx.tensor, offset=mx.offset,
                              ap=[list(mx.ap[0]), list(mx.ap[1]), [0, 8]])
                e_sb = p2one.tile([S, 8, 8], F32, tag="e_sb")
                nc.vector.tensor_tensor(e_sb, s_sb, mxb, op=ALU.subtract)
                ex_sb = p2one.tile([S, 8, 8], F32, tag="ex_sb")
                nc.scalar.activation(ex_sb, e_sb, AF.Exp)
                sm = p2one.tile([S, 8], F32, tag="sm")
                nc.vector.reduce_sum(sm, ex_sb, axis=mybir.AxisListType.X)
                rs = p2one.tile([S, 8], F32, tag="rs")
                nc.vector.reciprocal_approx_fast(out=rs, in_=sm)
                rsb = bass.AP(tensor=rs.tensor, offset=rs.offset,
                              ap=[list(rs.ap[0]), list(rs.ap[1]), [0, 8]])
                attn = p2one.tile([S, S], F32R, tag="attn")
                nc.vector.tensor_tensor(attn.rearrange("p (a b) -> p a b", a=8), ex_sb, rsb, op=ALU.mult)
                maskbd = p2one.tile([S, S], F32R, tag="maskbd")
                nc.sync.dma_start(out=maskbd, in_=mask_in[:, :])
                attn_m = p2one.tile([S, S], F32R, tag="attn_m")
                nc.vector.tensor_tensor(attn_m, attn, maskbd, op=ALU.mult)
                identr = p2one.tile([S, S], F32R, tag="identr")
                nc.sync.dma_start(out=identr, in_=identr_in[:, :])
                attn_tp = p2ps.tile([S, S], F32R, tag="attn_tp", bufs=1)
                nc.tensor.transpose(attn_tp, attn_m, identr)
                attn_t = p2one.tile([S, S], BF16, tag="attn_t")
                nc.scalar.copy(attn_t, attn_tp)
                if debug_taps:
                    at_f = p2one.tile([S, S + 2], F32, tag="at_f")
                    nc.vector.tensor_copy(at_f[:, 0:S], attn_m[:, :].bitcast(F32))
                    nc.vector.tensor_copy(at_f[:, S:S + 1], rqs)
                    nc.vector.tensor_copy(at_f[:, S + 1:S + 2], rks)
                    nc.sync.dma_start(out=dbg_at[:, :], in_=at_f)
                wpo = p2one.tile([S, S], BF16, tag="wpo")
                nc.sync.dma_start(out=wpo, in_=wpo_in[:, :])
                for blk in range(HW // NPX):
                    ps_o = p2ps.tile([S, NPX], F32, tag="ps_o", bufs=2)
                    nc.tensor.matmul(ps_o, attn_t, vo_store[0:S, blk * NPX:(blk + 1) * NPX],
                                     start=True, stop=True)
                    o_sb = p2.tile([S, NPX], BF16, tag="o_sb")
                    nc.scalar.copy(o_sb, ps_o)
                    ps_po = p2ps.tile([S, NPX], F32, tag="ps_po", bufs=1)
                    nc.tensor.matmul(ps_po, wpo, o_sb, start=True, stop=True)
                    nc.vector.tensor_copy(vo_store[64:128, blk * NPX:(blk + 1) * NPX], ps_po)
                if debug_taps:
                    for half in range(2):
                        qk_f = p2.tile([128, HW // 2], F32, tag="qk_f", bufs=1)
                        nc.vector.tensor_copy(qk_f, qk_store[:, half * HW // 2:(half + 1) * HW // 2])
                        nc.sync.dma_start(out=dbg_qk[:, half * HW // 2:(half + 1) * HW // 2], in_=qk_f)
                        vo_f = p2.tile([128, HW // 2], F32, tag="vo_f", bufs=1)
                        nc.vector.tensor_copy(vo_f, vo_store[:, half * HW // 2:(half + 1) * HW // 2])
                        nc.sync.dma_start(out=dbg_vo[:, half * HW // 2:(half + 1) * HW // 2], in_=vo_f)

            # ================= PHASE 3: expand + LN + FFN =================
            with tc.tile_pool(name="p3w", bufs=1) as p3w, \
                 tc.tile_pool(name="p3", bufs=2) as p3, \
                 tc.tile_pool(name="p3ps", bufs=1, space="PSUM") as p3ps:
                wexpa = p3w.tile([128, 3, C], BF16, tag="wexpa")
                nc.sync.dma_start(out=wexpa, in_=wexpa_in[:, :, :])
                wexps = p3w.tile([S, 3, C], BF16, tag="wexps")
                nc.sync.dma_start(out=wexps, in_=wexps_in[:, :, :])
                wf1v = p3w.tile([128, 2, 2, 128], BF16, tag="wf1v")
                nc.sync.dma_start(out=wf1v, in_=wf1v_in[:, :, :, :])
                wdw = p3w.tile([128, 2, 9, 128], BF16, tag="wdw")
                nc.sync.dma_start(out=wdw, in_=wdw_in[:, :, :, :])
                wf2 = p3w.tile([128, 2, 2, 128], BF16, tag="wf2")
                nc.sync.dma_start(out=wf2, in_=wf2_in[:, :, :, :])
                stat256 = p3w.tile([128, 2], BF16, tag="stat256")
                nc.sync.dma_start(out=stat256, in_=stat256_in[:, :])
                ones128 = p3w.tile([1, 128], BF16, tag="ones128")
                nc.sync.dma_start(out=ones128, in_=ones128_in[:, :])
                bias_g = p3w.tile([128, 2, 1], F32, tag="bias_g")
                nc.sync.dma_start(out=bias_g, in_=bias_g_in[:, :, :])
                # owin rows 0:64: slot i = o row (r0-2)+i ; rows 64:128: slot i = o row (r0-2)+i+1
                # slot18 zero
                owin = p3w.tile([128, 19, WP], BF16, tag="owin")
                nc.sync.dma_start(out=owin.rearrange("p a b -> p (a b)"), in_=zerosb_in[:, :19 * WP])
                # f1win: slot i = f1 row (r0-3)+i (slots 0..18); slot19 always zero
                f1win = p3w.tile([128, 2, 20, WP], BF16, tag="f1win")
                for half in range(2):
                    nc.sync.dma_start(out=f1win[:, half].rearrange("p a b -> p (a b)"),
                                      in_=zerosb_in[:, :20 * WP])

                def stage_a(re, nrows, slo, b_i, s_i):
                    """expand conv rows re..re+nrows-1 (owin slot of row re = slo) + LN + ffn1 -> f1win"""
                    npx_e = nrows * W
                    ps_e0 = p3ps.tile([128, NPX], F32, tag="ps_e0")
                    ps_e1 = p3ps.tile([128, NPX], F32, tag="ps_e1")
                    for dxi in range(3):
                        dx = dxi - 1
                        # stacked pair: lower rows = o[re-1+..] (dy=-1), upper = o[re+..] (dy=0)
                        rhs_p = owin[:, slo - 1:slo - 1 + nrows, 1 + dx:1 + dx + W]
                        nc.tensor.matmul(ps_e0[:, 0:npx_e], wexpa[:, dxi, 0:128], rhs_p,
                                         start=(dxi == 0), stop=False)
                        nc.tensor.matmul(ps_e1[:, 0:npx_e], wexpa[:, dxi, 128:256], rhs_p,
                                         start=(dxi == 0), stop=False)
                    for dxi in range(3):
                        dx = dxi - 1
                        rhs_s = owin[0:64, slo + 1:slo + 1 + nrows, 1 + dx:1 + dx + W]
                        nc.tensor.matmul(ps_e0[:, 0:npx_e], wexps[:, dxi, 0:128], rhs_s,
                                         start=False, stop=(dxi == 2))
                        nc.tensor.matmul(ps_e1[:, 0:npx_e], wexps[:, dxi, 128:256], rhs_s,
                                         start=False, stop=(dxi == 2))
                    t0 = p3.tile([128, NPX], BF16, tag="t0")
                    t1 = p3.tile([128, NPX], BF16, tag="t1")
                    nc.scalar.copy(t0[:, 0:npx_e], ps_e0[:, 0:npx_e])
                    nc.scalar.copy(t1[:, 0:npx_e], ps_e1[:, 0:npx_e])
                    sq0 = p3.tile([128, NPX], BF16, tag="sq0")
                    sq1t = p3.tile([128, NPX], BF16, tag="sq1t")
                    nc.scalar.activation(sq0[:, 0:npx_e], ps_e0[:, 0:npx_e], AF.Square)
                    nc.scalar.activation(sq1t[:, 0:npx_e], ps_e1[:, 0:npx_e], AF.Square)
                    ps_stm = p3ps.tile([1, NPX], F32, tag="small", bufs=2)
                    nc.tensor.matmul(ps_stm[0:1, 0:npx_e], stat256[:, 0:1], t0[:, 0:npx_e], start=True, stop=False)
                    nc.tensor.matmul(ps_stm[0:1, 0:npx_e], stat256[:, 0:1], t1[:, 0:npx_e], start=False, stop=True)
                    ps_sts = p3ps.tile([1, NPX], F32, tag="small", bufs=2)
                    nc.tensor.matmul(ps_sts[0:1, 0:npx_e], stat256[:, 1:2], sq0[:, 0:npx_e], start=True, stop=False)
                    nc.tensor.matmul(ps_sts[0:1, 0:npx_e], stat256[:, 1:2], sq1t[:, 0:npx_e], start=False, stop=True)
                    mu3 = p3.tile([1, NPX], BF16, tag="mu3", bufs=1)
                    nc.scalar.copy(mu3[:, 0:npx_e], ps_stm[0:1, 0:npx_e])
                    musq = p3.tile([1, NPX], F32, tag="musq", bufs=1)
                    mu3v = mu3[:, 0:npx_e]
                    nc.vector.tensor_tensor(musq[:, 0:npx_e], mu3v, mu3v, op=ALU.mult)
                    varr = p3.tile([1, NPX], F32, tag="varr", bufs=1)
                    nc.vector.scalar_tensor_tensor(varr[:, 0:npx_e], ps_sts[0:1, 0:npx_e], EPS,
                                                   musq[:, 0:npx_e], op0=ALU.add, op1=ALU.subtract)
                    rcpv = p3.tile([1, NPX], F32, tag="rcpv", bufs=1)
                    nc.vector.reciprocal_approx_fast(out=rcpv[:, 0:npx_e], in_=varr[:, 0:npx_e])
                    r3 = p3.tile([1, NPX], BF16, tag="r3", bufs=1)
                    nc.scalar.activation(r3[:, 0:npx_e], rcpv[:, 0:npx_e], AF.Sqrt)
                    ps_mu = p3ps.tile([128, NPX], F32, tag="small", bufs=2)
                    nc.tensor.matmul(ps_mu[:, 0:npx_e], ones128, mu3[:, 0:npx_e], start=True, stop=True)
                    ps_r = p3ps.tile([128, NPX], F32, tag="small", bufs=2)
                    nc.tensor.matmul(ps_r[:, 0:npx_e], ones128, r3[:, 0:npx_e], start=True, stop=True)
                    vn0 = p3.tile([128, NPX], BF16, tag="vn0")
                    vn1 = p3.tile([128, NPX], BF16, tag="vn1")
                    for vt, tt in ((vn0, t0), (vn1, t1)):
                        dsb = p3.tile([128, NPX], F32, tag="dsb")
                        nc.vector.tensor_tensor(dsb[:, 0:npx_e], tt[:, 0:npx_e], ps_mu[:, 0:npx_e], op=ALU.subtract)
                        nc.vector.tensor_tensor(vt[:, 0:npx_e], dsb[:, 0:npx_e], ps_r[:, 0:npx_e], op=ALU.mult)
                    # ffn1-v + fx -> f1win rows re.. (slot = re-(r0-3) = slo+1)
                    for mt in range(2):
                        ps_f = p3ps.tile([128, NPX], F32, tag="ps_f")
                        nc.tensor.matmul(ps_f[:, 0:npx_e], wf1v[:, 0, mt], vn0[:, 0:npx_e], start=True, stop=False)
                        nc.tensor.matmul(ps_f[:, 0:npx_e], wf1v[:, 1, mt], vn1[:, 0:npx_e], start=False, stop=True)
                        fxs = p3.tile([128, NPX], BF16, tag="fxs")
                        nc.sync.dma_start(out=fxs[:, 0:npx_e], in_=fx_dram[mt, :, re * W:re * W + npx_e])
                        dstf = f1win[:, mt, slo + 1:slo + 1 + nrows, 1:1 + W]
                        nc.vector.tensor_tensor(dstf,
                                                ps_f[:, 0:npx_e].rearrange("p (a b) -> p a b", a=nrows),
                                                fxs[:, 0:npx_e].rearrange("p (a b) -> p a b", a=nrows),
                                                op=ALU.add)

                def stage_b(rg, nrg, slg):
                    """dw conv rows rg..rg+nrg-1 (f1win slot of row rg = slg) + gelu + ffn2 -> out"""
                    npx_g = nrg * W
                    gsb = p3.tile([128, 2, NPX], BF16, tag="gsb")
                    for ct in range(2):
                        ps_g = p3ps.tile([128, NPX], F32, tag="ps_g")
                        for t_i, (dy, dx) in enumerate(TAPS):
                            sl0 = slg + dy
                            rhs = f1win[:, ct, sl0:sl0 + nrg, 1 + dx:1 + dx + W]
                            nc.tensor.matmul(ps_g[:, 0:npx_g], wdw[:, ct, t_i], rhs,
                                             start=(t_i == 0), stop=(t_i == 8))
                        nc.scalar.activation(gsb[:, ct, 0:npx_g], ps_g[:, 0:npx_g], AF.Gelu,
                                             bias=bias_g[:, ct])
                    for mt in range(2):
                        ps_out = p3ps.tile([128, NPX], F32, tag="ps_out")
                        nc.tensor.matmul(ps_out[:, 0:npx_g], wf2[:, 0, mt], gsb[:, 0, 0:npx_g], start=True, stop=False)
                        nc.tensor.matmul(ps_out[:, 0:npx_g], wf2[:, 1, mt], gsb[:, 1, 0:npx_g], start=False, stop=True)
                        osb = p3.tile([128, NPX], BF16, tag="osb")
                        nc.scalar.copy(osb[:, 0:npx_g], ps_out[:, 0:npx_g])
                        nc.sync.dma_start(out=out_dram[mt, :, rg * W:rg * W + npx_g], in_=osb[:, 0:npx_g])

                for s_i in range(NSTRIP):
                    r0 = 16 * s_i
                    if s_i > 0:
                        nc.vector.tensor_copy(owin[:, 0:2], owin[:, 16:18])
                        nc.vector.tensor_copy(f1win[:, :, 0:3], f1win[:, :, 16:19])
                    # stage A over blocks
                    for b_i in range(NBLK):
                        rb = r0 + BLK_ROWS * b_i
                        osrc = vo_store[64:128, rb * W:(rb + 4) * W].rearrange("p (a b) -> p a b", a=4)
                        nc.vector.tensor_copy(owin[0:64, 4 * b_i + 2:4 * b_i + 6, 1:1 + W], osrc)
                        nc.vector.tensor_copy(owin[64:128, 4 * b_i + 1:4 * b_i + 5, 1:1 + W], osrc)
                        if s_i == 0 and b_i == 0:
                            stage_a(0, 3, 2, b_i, s_i)
                        else:
                            stage_a(rb - 1, 4, 4 * b_i + 1, b_i, s_i)
                    if s_i == NSTRIP - 1:
                        # f1 row 127 epilogue (o rows 126..128 ; owin slot of row 126 = 16)
                        stage_a(127, 1, 17, 0, s_i)
                    # stage B over blocks (rows r0-2 .. r0+13)
                    for b_i in range(NBLK):
                        rb = r0 + BLK_ROWS * b_i
                        if s_i == 0 and b_i == 0:
                            stage_b(0, 2, 3)
                        else:
                            stage_b(rb - 2, 4, 4 * b_i + 1)
                # out rows 126,127 (f1win slot of row 126 = 126-109 = 17 ; slot19 zero? need row 128->slot19)
                stage_b(126, 2, 17)
    return nc


def _prep_host(inputs):
    f32 = np.float32
    w_cq = np.asarray(inputs["w_cq"], f32)
    w_ckv = np.asarray(inputs["w_ckv"], f32)
    ln_q_w = np.asarray(inputs["ln_q_w"], f32); ln_q_b = np.asarray(inputs["ln_q_b"], f32)
    ln_kv_w = np.asarray(inputs["ln_kv_w"], f32); ln_kv_b = np.asarray(inputs["ln_kv_b"], f32)
    w_kv = np.asarray(inputs["w_kv"], f32)
    w_kvdw = np.asarray(inputs["w_kvdw"], f32)
    w_q = np.asarray(inputs["w_q"], f32)
    temperature = np.asarray(inputs["temperature"], f32)
    w_po = np.asarray(inputs["w_po"], f32)
    w_expand = np.asarray(inputs["w_expand"], f32)
    ln_out_w = np.asarray(inputs["ln_out_w"], f32); ln_out_b = np.asarray(inputs["ln_out_b"], f32)
    w_ffn1 = np.asarray(inputs["w_ffn1"], f32)
    w_ffn_dw = np.asarray(inputs["w_ffn_dw"], f32)
    w_ffn2 = np.asarray(inputs["w_ffn2"], f32)

    d = {}
    bf = ml_dtypes.bfloat16
    def conv_lhsT(wc):
        a = np.zeros((128, 2, 9, S + 1), f32)
        for kt in range(2):
            blk = wc[:, kt * 128:(kt + 1) * 128]           # [S, 128, 3, 3]
            a[:, kt, :, :S] = blk.transpose(1, 2, 3, 0).reshape(128, 9, S)
            a[:, kt, :, S] = blk.sum(axis=0).reshape(128, 9)
        return a.astype(bf)
    d["wcq"] = conv_lhsT(w_cq)
    d["wckv"] = conv_lhsT(w_ckv)
    w_q_eff = w_q * ln_q_w[None, :, None, None]
    d["bias_q"] = (w_q * ln_q_b[None, :, None, None]).sum(axis=(1, 2, 3)).reshape(S, 1)
    wqkv = np.zeros((128, 9, 128), f32)
    wqkv[0:64, :, 0:S] = w_q_eff.transpose(1, 2, 3, 0).reshape(S, 9, S)
    w_kv_g = w_kv[:, :, 0, 0] * ln_kv_w[None, :]
    w_kv_eff = w_kvdw[:, 0][:, None] * w_kv_g[:, :, None, None]   # [2S, S, 3, 3]
    d["bias_kv"] = (w_kvdw[:, 0].sum(axis=(1, 2)) * (w_kv[:, :, 0, 0] @ ln_kv_b)).reshape(2 * S, 1)
    wqkv[64:128, :, :] = w_kv_eff.transpose(1, 2, 3, 0).reshape(S, 9, 2 * S)
    d["wqkv"] = wqkv.astype(bf)
    d["wpo"] = np.ascontiguousarray(w_po[:, :, 0, 0].T).astype(bf)
    # expand conv, dual-tap stacked: wexpa rows 0:64 = tap(dy=-1,dx), rows 64:128 = tap(dy=0,dx)
    wexpa = np.zeros((128, 3, C), f32)
    wexps = np.zeros((S, 3, C), f32)
    for dxi in range(3):
        wexpa[0:64, dxi] = w_expand[:, :, 0, dxi].T
        wexpa[64:128, dxi] = w_expand[:, :, 1, dxi].T
        wexps[:, dxi] = w_expand[:, :, 2, dxi].T
    d["wexpa"] = wexpa.astype(bf)
    d["wexps"] = wexps.astype(bf)
    w1 = w_ffn1[:, :, 0, 0]
    w1x = w1[:, :C]
    w1v = w1[:, C:] * ln_out_w[None, :]
    def one_by_one_lhsT(wm):
        a = np.zeros((128, 2, 2, 128), f32)
        for kt in range(2):
            for mt in range(2):
                a[:, kt, mt, :] = wm[mt * 128:(mt + 1) * 128, kt * 128:(kt + 1) * 128].T
        return a
    d["wf1x"] = one_by_one_lhsT(w1x).astype(bf)
    d["wf1v"] = one_by_one_lhsT(w1v).astype(bf)
    bias_f1 = w1[:, C:] @ ln_out_b
    dw_t = w_ffn_dw[:, 0].reshape(C, 9)
    d["bias_g"] = np.ascontiguousarray(
        (bias_f1 * dw_t.sum(1)).reshape(2, 128, 1).transpose(1, 0, 2))
    wdw = np.zeros((128, 2, 9, 128), f32)
    for ct in range(2):
        for t in range(9):
            np.fill_diagonal(wdw[:, ct, t, :], dw_t[ct * 128:(ct + 1) * 128, t])
    d["wdw"] = wdw.astype(bf)
    d["wf2"] = one_by_one_lhsT(w_ffn2[:, :, 0, 0]).astype(bf)
    stat_cq = np.zeros((S + 1, 1), f32)
    stat_cq[:S, 0] = 1.0 / S
    stat_cq[S, 0] = -1.0 / (S * S)
    d["stat_cq"] = stat_cq.astype(bf)
    bc2a = np.zeros((1, 128), f32); bc2a[0, 0:64] = 1.0
    bc2b = np.zeros((1, 128), f32); bc2b[0, 64:128] = 1.0
    d["bc2a"] = bc2a.astype(bf); d["bc2b"] = bc2b.astype(bf)
    d["ones128"] = np.ones((1, 128), f32).astype(bf)
    stat256 = np.zeros((128, 2), f32)
    stat256[:, 0] = 1.0 / C
    stat256[:, 1] = 1.0 / C
    d["stat256"] = stat256.astype(bf)
    d["tempv"] = np.repeat(temperature.reshape(HEADS), S // HEADS).reshape(S, 1).astype(f32)
    mask = np.zeros((S, S), f32)
    for h in range(HEADS):
        mask[h * 8:(h + 1) * 8, h * 8:(h + 1) * 8] = 1.0
    d["maskbd"] = mask
    d["ident"] = np.eye(128, dtype=f32).astype(bf)
    d["identr"] = np.eye(S, dtype=f32)
    d["onesr"] = np.ones((1, S), f32)
    d["zerosb"] = np.zeros((128, 3000), f32).astype(bf)
    return d


def _pad_input(x):
    """[C,H,W] f32 -> [128, 2, H+4, WP] bf16 zero-padded, partition-major"""
    out = np.zeros((128, 2, H + 4, WP), ml_dtypes.bfloat16)
    out[:, :, 2:H + 2, 1:W + 1] = x.reshape(2, 128, H, W).transpose(1, 0, 2, 3).astype(ml_dtypes.bfloat16)
    return out


def kernel(**inputs):
    key = "nc"
    if key not in _CACHED:
        nc = build_nc(debug_taps=False)
        nc.finalize()
        _CACHED[key] = nc
    nc = _CACHED[key]
    d = _prep_host(inputs)
    x = np.asarray(inputs["x"], np.float32)
    y = np.asarray(inputs["y"], np.float32)
    in_maps = []
    for i in range(B):
        m = dict(d)
        m["x"] = _pad_input(x[i])
        m["y"] = _pad_input(y[i])
        in_maps.append(m)
    res = run_bass_kernel_spmd(nc, in_maps, list(range(B)))
    out = np.stack([res.results[i]["out"].reshape(C, H, W) for i in range(B)])
    return out.astype(np.float32)

